# revision 19
# baseline (speedup 1.0000x reference)
"""Trainium2 Bass kernel for nn_LowRankExperts.

Reference computation (E=16 experts, B=1024, I=512, H=16, R=8, O=512,
F = I*R + R*O = 8192):
    h = tanh(einsum('bi,eih->ebh', x, W1) + b1)      # [E,B,H]
    f = einsum('ebh,ehf->ebf', h, W2) + b2           # [E,B,F]
    A  = f[..., :I*R].reshape(E,B,I,R)
    Bf = f[..., I*R:].reshape(E,B,R,O)
    return (A, Bf)

Sharding: expert-parallel, 2 experts per core on 8 cores; x replicated.
The output (512 MiB fp32) dominates: each core writes 64 MiB, so the
kernel is output-DMA-bound (~360-440 GB/s per-core HBM write).

Key design points:
  - Matmuls run in fp32r (fp32 with 11-bit mantissa, 1 cycle/row on the
    PE vs 4 for plain fp32). Operands are pre-rounded on the host.
  - x is pre-transposed on the host to xT [I,B] packed as [128, 4, B]
    (I on partitions) so GEMM1 needs no on-chip transpose.
  - GEMM2's contraction is only K=17 (16 h rows + a ones row that pairs
    with a b2 row folded into the W2 operand, adding the bias for free).
    A 17-partition SBUF tensor would DMA-load through a single SDMA
    engine (~26 GB/s), so W2 is packed into FOUR row groups at partition
    bases 0/32/64/96, each holding a quarter of F -- a full-width fast
    DMA -- and GEMM2 issues row-group matmuls (tile_position=(32g,0)).
  - GEMM1 replicates hT into all 4 bases at no PE cost by replicating
    W1's output columns (M=128 instead of 16; matmul time is set by the
    moving dim only). Tanh+b1 is applied by ACT straight out of PSUM,
    once per row group.
  - PSUM [128,512] tiles -> SBUF via DVE/ACT copies (split 1:1) ->
    1 MiB HWDGE DMAs into the contiguous A / Bf halves.
"""

import sys

import numpy as np

try:
    import concourse  # noqa: F401
except ImportError:
    sys.path.insert(0, "/opt/trn_rl_repo")

E, B, I, O, R, H = 16, 1024, 512, 512, 8, 16
F = I * R + R * O  # 8192
N_CORES = 8
EL = E // N_CORES  # experts per core = 2
P = 128
KP = H + 1  # GEMM2 contraction rows per group: 16 h rows + 1 ones/bias row
NG = 4  # row groups (partition bases 0/32/64/96)
FG = F // NG  # 2048 F columns per row group
FC = 512  # matmul moving-dim chunk (one PSUM bank of fp32)
NC_G = FG // FC  # 4 chunks per group
NB = B // P  # 8 b-chunks
K1 = I // P  # 4 contraction chunks for GEMM1
FH = F // 2  # 4096, size of the A / Bf halves

_nc_cache = None


def _build_nc():
    import concourse.bacc as bacc
    import concourse.mybir as mybir
    import concourse.tile as tile

    f32 = mybir.dt.float32
    f32r = mybir.dt.float32r  # fp32 @ 1 cycle/row on the PE (11-bit mantissa)

    nc = bacc.Bacc(
        "TRN2",
        target_bir_lowering=False,
        debug=False,
        enable_asserts=False,
        num_devices=N_CORES,
    )

    xp_d = nc.dram_tensor("xp", (P, K1 * B), f32r, kind="ExternalInput")
    w1_d = nc.dram_tensor("w1p", (P, EL * K1 * P), f32r, kind="ExternalInput")
    b1_d = nc.dram_tensor("b1p", (P, EL), f32, kind="ExternalInput")
    w2_d = nc.dram_tensor("w2p", (P, EL * FG), f32r, kind="ExternalInput")
    pad_d = nc.dram_tensor("padp", (1, EL * B), f32r, kind="ExternalInput")
    a_d = nc.dram_tensor("a_out", (EL, B, FH), f32, kind="ExternalOutput")
    bf_d = nc.dram_tensor("bf_out", (EL, B, FH), f32, kind="ExternalOutput")

    with tile.TileContext(nc) as tc:
        with (
            tc.tile_pool(name="consts", bufs=1) as cpool,
            tc.tile_pool(name="fb", bufs=10) as fpool,
            tc.tile_pool(name="psum", bufs=8, space="PSUM") as pspool,
        ):
            # All input loads are full-width (128-partition) fast DMAs on
            # the sync ring, ordered so GEMM1 can start as early as possible.
            w1 = cpool.tile([P, EL, K1, P], f32r)
            nc.sync.dma_start(
                w1[:], w1_d.ap().rearrange("p (e k m) -> p e k m", e=EL, k=K1)
            )
            b1s = cpool.tile([P, EL], f32)
            nc.sync.dma_start(b1s[:], b1_d.ap())
            xp_r = xp_d.ap().rearrange("p (k b) -> p k b", k=K1)
            xt = cpool.tile([P, K1, B], f32r)
            for k in range(K1):
                nc.sync.dma_start(xt[:, k], xp_r[:, k])
            w2 = cpool.tile([P, EL, FG], f32r)
            w2_r = w2_d.ap().rearrange("p (e f) -> p e f", e=EL)
            for e in range(EL):
                nc.sync.dma_start(w2[:, e], w2_r[:, e])

            ht = cpool.tile([P, EL, B], f32r)
            # Ones row at partition 32g+16 of each group (pairs with the b2
            # row of w2). Engine APs need 32-aligned partition bases, so
            # fill these single partitions via tiny DMAs (scalar ring, so
            # they don't queue behind the bulk loads on the sync ring).
            pad_r = pad_d.ap().rearrange("o (e b) -> o e b", e=EL)
            for g in range(NG):
                nc.scalar.dma_start(ht[32 * g + H : 32 * g + KP], pad_r)

            # GEMM1 + tanh: hT [16, B] per expert, replicated into the 4
            # row groups by w1's replicated output columns (single matmul
            # chain produces all 4 copies; ACT applies tanh+b1 per group).
            for e in range(EL):
                for n in range(B // FC):
                    ph = pspool.tile([P, FC], f32, tag="bank")
                    for k in range(K1):
                        nc.tensor.matmul(
                            ph[:],
                            w1[:, e, k, :],
                            xt[:, k, n * FC : (n + 1) * FC],
                            start=(k == 0),
                            stop=(k == K1 - 1),
                        )
                    for g in range(NG):
                        nc.scalar.activation(
                            ht[32 * g : 32 * g + H, e, n * FC : (n + 1) * FC],
                            ph[32 * g : 32 * g + H],
                            mybir.ActivationFunctionType.Tanh,
                            bias=b1s[32 * g : 32 * g + H, e : e + 1],
                        )

            # GEMM2: f[e, b-chunk] [128, F] via row-group matmuls; PSUM ->
            # SBUF copies split 1:1 over DVE/ACT; 1 MiB DMA per (group,
            # half-row-block) into the A (groups 0,1) / Bf (groups 2,3)
            # output halves.
            for e in range(EL):
                for b in range(NB):
                    for g in range(NG):
                        fb = fpool.tile([P, FG], f32, tag="fb")
                        for c in range(NC_G):
                            pf = pspool.tile([P, FC], f32, tag="bank")
                            nc.tensor.matmul(
                                pf[:],
                                ht[32 * g : 32 * g + KP, e, b * P : (b + 1) * P],
                                w2[32 * g : 32 * g + KP, e, c * FC : (c + 1) * FC],
                                start=True,
                                stop=True,
                                tile_position=(32 * g, 0),
                            )
                            if c % 2 == 1:
                                nc.scalar.copy(pf_dst(fb, c), pf[:])
                            else:
                                nc.vector.tensor_copy(pf_dst(fb, c), pf[:])
                        out_d = a_d if g < 2 else bf_d
                        col0 = (g % 2) * FG
                        nc.sync.dma_start(
                            out_d.ap()[
                                e, b * P : (b + 1) * P, col0 : col0 + FG
                            ],
                            fb[:],
                        )

    nc.compile()
    return nc


def pf_dst(fb, c):
    return fb[:, c * FC : (c + 1) * FC]


def _round_fp32r(a):
    """Round fp32 to the PE's fp32r format (11 explicit mantissa bits,
    round-to-nearest-even) — matches walrus fp32_to_fp32r."""
    a = np.ascontiguousarray(a, dtype=np.float32)
    u = a.view(np.uint32)
    bias = ((u >> 12) & 1) + np.uint32(0x7FF)
    u2 = (u + bias) & np.uint32(0xFFFFF000)
    return u2.view(np.float32)


def _prep_inputs(x, W1, b1, W2, b2):
    """Host-side packing into the per-core DMA-friendly layouts."""
    x = np.ascontiguousarray(x, dtype=np.float32)
    W1 = np.ascontiguousarray(W1, dtype=np.float32)
    b1 = np.ascontiguousarray(b1, dtype=np.float32)
    W2 = np.ascontiguousarray(W2, dtype=np.float32)
    b2 = np.ascontiguousarray(b2, dtype=np.float32)
    # xT packed [P, K1*B]: xp[p, k*B + b] = x[b, k*128 + p]
    xp = _round_fp32r(
        np.ascontiguousarray(
            x.T.reshape(K1, P, B).transpose(1, 0, 2).reshape(P, K1 * B)
        )
    )
    padp = np.ones((1, EL * B), dtype=np.float32)
    in_maps = []
    for c in range(N_CORES):
        e0 = c * EL
        # w1 with output columns replicated into the 4 row groups:
        # w1p[p, e, k, 32g+h] = W1[e0+e, k*128+p, h]; cols 16..31 of each
        # group are zero (their hT rows are never read by GEMM2).
        w1b = W1[e0 : e0 + EL].reshape(EL, K1, P, H).transpose(2, 0, 1, 3)
        w1r = np.zeros((P, EL, K1, NG, 32), dtype=np.float32)
        w1r[..., :H] = w1b[:, :, :, None, :]
        w1p = _round_fp32r(w1r.reshape(P, EL * K1 * P))
        # b1 replicated per group: b1p[32g+h, e] = b1[e0+e, h]
        b1p = np.zeros((P, EL), dtype=np.float32)
        b1p.reshape(NG, 32, EL)[:, :H] = b1[e0 : e0 + EL].T[None, :, :]
        # w2 packed into row groups: w2p[32g+k, e, j] = W2[e0+e, k, g*FG+j]
        # (k<16); row 32g+16 = b2[e0+e, g*FG+j]; rows 17..31 unused.
        w2v = W2[e0 : e0 + EL].reshape(EL, H, NG, FG)
        w2c = np.zeros((NG, 32, EL, FG), dtype=np.float32)
        w2c[:, :H] = w2v.transpose(2, 1, 0, 3)
        w2c[:, H] = b2[e0 : e0 + EL].reshape(EL, NG, FG).transpose(1, 0, 2)
        w2p = _round_fp32r(w2c.reshape(P, EL * FG))
        in_maps.append(
            {"xp": xp, "w1p": w1p, "b1p": b1p, "w2p": w2p, "padp": padp}
        )
    return in_maps


def kernel(x, W1, b1, W2, b2, _want_results=False, **run_kwargs):
    global _nc_cache
    from concourse.bass_utils import run_bass_kernel_spmd

    if _nc_cache is None:
        _nc_cache = _build_nc()
    nc = _nc_cache

    in_maps = _prep_inputs(x, W1, b1, W2, b2)
    try:
        res = run_bass_kernel_spmd(
            nc, in_maps, core_ids=list(range(N_CORES)), **run_kwargs
        )
    except Exception:
        # Rare transient NRT execution failures recover on re-execution.
        res = run_bass_kernel_spmd(
            nc, in_maps, core_ids=list(range(N_CORES)), **run_kwargs
        )
    A = np.concatenate(
        [res.results[c]["a_out"].reshape(EL, B, I, R) for c in range(N_CORES)], axis=0
    )
    Bf = np.concatenate(
        [res.results[c]["bf_out"].reshape(EL, B, R, O) for c in range(N_CORES)], axis=0
    )
    if _want_results:
        return (A, Bf), res
    return (A, Bf)


# revision 20
# speedup vs baseline: 1.2083x; 1.2083x over previous
"""Trainium2 Bass kernel for nn_LowRankExperts.

Reference computation (E=16 experts, B=1024, I=512, H=16, R=8, O=512,
F = I*R + R*O = 8192):
    h = tanh(einsum('bi,eih->ebh', x, W1) + b1)      # [E,B,H]
    f = einsum('ebh,ehf->ebf', h, W2) + b2           # [E,B,F]
    A  = f[..., :I*R].reshape(E,B,I,R)
    Bf = f[..., I*R:].reshape(E,B,R,O)
    return (A, Bf)

Sharding: expert-parallel, 2 experts per core on 8 cores; x replicated.
The output (512 MiB fp32) dominates: each core writes 64 MiB, so the
kernel is output-DMA-bound (~360-440 GB/s per-core HBM write).

Key design points:
  - Matmuls run in fp32r (fp32 with 11-bit mantissa, 1 cycle/row on the
    PE vs 4 for plain fp32). Operands are pre-rounded on the host.
  - x is pre-transposed on the host to xT [I,B] packed as [128, 4, B]
    (I on partitions) so GEMM1 needs no on-chip transpose.
  - GEMM2's contraction is only K=17 (16 h rows + a ones row that pairs
    with a b2 row folded into the W2 operand, adding the bias for free).
    A 17-partition SBUF tensor would DMA-load through a single SDMA
    engine (~26 GB/s), so W2 is packed into FOUR row groups at partition
    bases 0/32/64/96, each holding a quarter of F -- a full-width fast
    DMA -- and GEMM2 issues row-group matmuls (tile_position=(32g,0)).
  - GEMM1 replicates hT into all 4 bases at no PE cost by replicating
    W1's output columns (M=128 instead of 16; matmul time is set by the
    moving dim only). Tanh+b1 is applied by ACT straight out of PSUM,
    once per row group.
  - PSUM [128,512] tiles -> SBUF via DVE/ACT copies (split 1:1) ->
    1 MiB HWDGE DMAs into the contiguous A / Bf halves.
"""

import sys

import numpy as np

try:
    import concourse  # noqa: F401
except ImportError:
    sys.path.insert(0, "/opt/trn_rl_repo")

E, B, I, O, R, H = 16, 1024, 512, 512, 8, 16
F = I * R + R * O  # 8192
N_CORES = 8
EL = E // N_CORES  # experts per core = 2
P = 128
KP = H + 1  # GEMM2 contraction rows per group: 16 h rows + 1 ones/bias row
NG = 4  # row groups (partition bases 0/32/64/96)
FG = F // NG  # 2048 F columns per row group
FC = 512  # matmul moving-dim chunk (one PSUM bank of fp32)
NC_G = FG // FC  # 4 chunks per group
NB = B // P  # 8 b-chunks
K1 = I // P  # 4 contraction chunks for GEMM1
FH = F // 2  # 4096, size of the A / Bf halves

_nc_cache = None


def _build_nc():
    import concourse.bacc as bacc
    import concourse.mybir as mybir
    import concourse.tile as tile

    f32 = mybir.dt.float32
    f32r = mybir.dt.float32r  # fp32 @ 1 cycle/row on the PE (11-bit mantissa)

    nc = bacc.Bacc(
        "TRN2",
        target_bir_lowering=False,
        debug=False,
        enable_asserts=False,
        num_devices=N_CORES,
    )

    xp_d = nc.dram_tensor("xp", (P, K1 * B), f32r, kind="ExternalInput")
    w1_d = nc.dram_tensor("w1p", (P, EL * K1 * P), f32r, kind="ExternalInput")
    b1_d = nc.dram_tensor("b1p", (P, EL), f32, kind="ExternalInput")
    w2_d = nc.dram_tensor("w2p", (P, EL * FG), f32r, kind="ExternalInput")
    pad_d = nc.dram_tensor("padp", (1, EL * B), f32r, kind="ExternalInput")
    a_d = nc.dram_tensor("a_out", (EL, B, FH), f32, kind="ExternalOutput")
    bf_d = nc.dram_tensor("bf_out", (EL, B, FH), f32, kind="ExternalOutput")

    with tile.TileContext(nc) as tc:
        with (
            tc.tile_pool(name="consts", bufs=1) as cpool,
            tc.tile_pool(name="fb", bufs=10) as fpool,
            tc.tile_pool(name="psum", bufs=8, space="PSUM") as pspool,
        ):
            # All input loads are full-width (128-partition) fast DMAs on
            # the sync ring, ordered so GEMM1 can start as early as possible.
            w1 = cpool.tile([P, EL, K1, P], f32r)
            w1_r = w1_d.ap().rearrange("p (e k m) -> p e k m", e=EL, k=K1)
            for e in range(EL):
                nc.sync.dma_start(w1[:, e], w1_r[:, e])
            b1s = cpool.tile([P, EL], f32)
            nc.sync.dma_start(b1s[:], b1_d.ap())
            xp_r = xp_d.ap().rearrange("p (k b) -> p k b", k=K1)
            xt = cpool.tile([P, K1, B], f32r)
            for k in range(K1):
                nc.sync.dma_start(xt[:, k], xp_r[:, k])
            w2 = cpool.tile([P, EL, FG], f32r)
            w2_r = w2_d.ap().rearrange("p (e f) -> p e f", e=EL)
            for e in range(EL):
                nc.sync.dma_start(w2[:, e], w2_r[:, e])

            ht = cpool.tile([P, EL, B], f32r)
            # Ones row at partition 32g+16 of each group (pairs with the b2
            # row of w2). Engine APs need 32-aligned partition bases, so
            # fill these single partitions via tiny DMAs (scalar ring, so
            # they don't queue behind the bulk loads on the sync ring).
            pad_r = pad_d.ap().rearrange("o (e b) -> o e b", e=EL)
            for g in range(NG):
                nc.scalar.dma_start(ht[32 * g + H : 32 * g + KP], pad_r)

            # GEMM1 + tanh: hT [16, B] per expert, replicated into the 4
            # row groups by w1's replicated output columns (single matmul
            # chain produces all 4 copies; ACT applies tanh+b1 per group).
            def gemm1(e, n):
                ph = pspool.tile([P, FC], f32, tag="bank", name=f"ph_{e}_{n}")
                for k in range(K1):
                    nc.tensor.matmul(
                        ph[:],
                        w1[:, e, k, :],
                        xt[:, k, n * FC : (n + 1) * FC],
                        start=(k == 0),
                        stop=(k == K1 - 1),
                    )
                for g in range(NG):
                    nc.scalar.activation(
                        ht[32 * g : 32 * g + H, e, n * FC : (n + 1) * FC],
                        ph[32 * g : 32 * g + H],
                        mybir.ActivationFunctionType.Tanh,
                        bias=b1s[32 * g : 32 * g + H, e : e + 1],
                    )

            # GEMM2: f[e, b-chunk] [128, F] via row-group matmuls; PSUM ->
            # SBUF copies split 1:1 over DVE/ACT; 1 MiB DMA per (group,
            # half-row-block) into the A (groups 0,1) / Bf (groups 2,3)
            # output halves. GEMM1 for each B-half is emitted just before
            # the b-chunks that consume it, so the first output tile's
            # chain is as short as possible and later GEMM1 work fills PE
            # gaps during GEMM2.
            for e in range(EL):
                for b in range(NB):
                    if b % (FC // P) == 0:
                        gemm1(e, b // (FC // P))
                    for g in range(NG):
                        fb = fpool.tile([P, FG], f32, tag="fb")
                        for c in range(NC_G):
                            pf = pspool.tile([P, FC], f32, tag="bank")
                            nc.tensor.matmul(
                                pf[:],
                                ht[32 * g : 32 * g + KP, e, b * P : (b + 1) * P],
                                w2[32 * g : 32 * g + KP, e, c * FC : (c + 1) * FC],
                                start=True,
                                stop=True,
                                tile_position=(32 * g, 0),
                            )
                            if c % 2 == 1:
                                nc.scalar.copy(pf_dst(fb, c), pf[:])
                            else:
                                nc.vector.tensor_copy(pf_dst(fb, c), pf[:])
                        out_d = a_d if g < 2 else bf_d
                        col0 = (g % 2) * FG
                        nc.sync.dma_start(
                            out_d.ap()[
                                e, b * P : (b + 1) * P, col0 : col0 + FG
                            ],
                            fb[:],
                        )

    nc.compile()
    return nc


def pf_dst(fb, c):
    return fb[:, c * FC : (c + 1) * FC]


def _round_fp32r(a):
    """Round fp32 to the PE's fp32r format (11 explicit mantissa bits,
    round-to-nearest-even) — matches walrus fp32_to_fp32r."""
    a = np.ascontiguousarray(a, dtype=np.float32)
    u = a.view(np.uint32)
    bias = ((u >> 12) & 1) + np.uint32(0x7FF)
    u2 = (u + bias) & np.uint32(0xFFFFF000)
    return u2.view(np.float32)


def _prep_inputs(x, W1, b1, W2, b2):
    """Host-side packing into the per-core DMA-friendly layouts."""
    x = np.ascontiguousarray(x, dtype=np.float32)
    W1 = np.ascontiguousarray(W1, dtype=np.float32)
    b1 = np.ascontiguousarray(b1, dtype=np.float32)
    W2 = np.ascontiguousarray(W2, dtype=np.float32)
    b2 = np.ascontiguousarray(b2, dtype=np.float32)
    # xT packed [P, K1*B]: xp[p, k*B + b] = x[b, k*128 + p]
    xp = _round_fp32r(
        np.ascontiguousarray(
            x.T.reshape(K1, P, B).transpose(1, 0, 2).reshape(P, K1 * B)
        )
    )
    padp = np.ones((1, EL * B), dtype=np.float32)
    in_maps = []
    for c in range(N_CORES):
        e0 = c * EL
        # w1 with output columns replicated into the 4 row groups:
        # w1p[p, e, k, 32g+h] = W1[e0+e, k*128+p, h]; cols 16..31 of each
        # group are zero (their hT rows are never read by GEMM2).
        w1b = W1[e0 : e0 + EL].reshape(EL, K1, P, H).transpose(2, 0, 1, 3)
        w1r = np.zeros((P, EL, K1, NG, 32), dtype=np.float32)
        w1r[..., :H] = w1b[:, :, :, None, :]
        w1p = _round_fp32r(w1r.reshape(P, EL * K1 * P))
        # b1 replicated per group: b1p[32g+h, e] = b1[e0+e, h]
        b1p = np.zeros((P, EL), dtype=np.float32)
        b1p.reshape(NG, 32, EL)[:, :H] = b1[e0 : e0 + EL].T[None, :, :]
        # w2 packed into row groups: w2p[32g+k, e, j] = W2[e0+e, k, g*FG+j]
        # (k<16); row 32g+16 = b2[e0+e, g*FG+j]; rows 17..31 unused.
        w2v = W2[e0 : e0 + EL].reshape(EL, H, NG, FG)
        w2c = np.zeros((NG, 32, EL, FG), dtype=np.float32)
        w2c[:, :H] = w2v.transpose(2, 1, 0, 3)
        w2c[:, H] = b2[e0 : e0 + EL].reshape(EL, NG, FG).transpose(1, 0, 2)
        w2p = _round_fp32r(w2c.reshape(P, EL * FG))
        in_maps.append(
            {"xp": xp, "w1p": w1p, "b1p": b1p, "w2p": w2p, "padp": padp}
        )
    return in_maps


def kernel(x, W1, b1, W2, b2, _want_results=False, **run_kwargs):
    global _nc_cache
    from concourse.bass_utils import run_bass_kernel_spmd

    if _nc_cache is None:
        _nc_cache = _build_nc()
    nc = _nc_cache

    in_maps = _prep_inputs(x, W1, b1, W2, b2)
    try:
        res = run_bass_kernel_spmd(
            nc, in_maps, core_ids=list(range(N_CORES)), **run_kwargs
        )
    except Exception:
        # Rare transient NRT execution failures recover on re-execution.
        res = run_bass_kernel_spmd(
            nc, in_maps, core_ids=list(range(N_CORES)), **run_kwargs
        )
    A = np.concatenate(
        [res.results[c]["a_out"].reshape(EL, B, I, R) for c in range(N_CORES)], axis=0
    )
    Bf = np.concatenate(
        [res.results[c]["bf_out"].reshape(EL, B, R, O) for c in range(N_CORES)], axis=0
    )
    if _want_results:
        return (A, Bf), res
    return (A, Bf)


# revision 21
# speedup vs baseline: 1.2129x; 1.0038x over previous
"""Trainium2 Bass kernel for nn_LowRankExperts.

Reference computation (E=16 experts, B=1024, I=512, H=16, R=8, O=512,
F = I*R + R*O = 8192):
    h = tanh(einsum('bi,eih->ebh', x, W1) + b1)      # [E,B,H]
    f = einsum('ebh,ehf->ebf', h, W2) + b2           # [E,B,F]
    A  = f[..., :I*R].reshape(E,B,I,R)
    Bf = f[..., I*R:].reshape(E,B,R,O)
    return (A, Bf)

Sharding: expert-parallel, 2 experts per core on 8 cores; x replicated.
The output (512 MiB fp32) dominates: each core writes 64 MiB, so the
kernel is output-DMA-bound (~360-440 GB/s per-core HBM write).

Key design points:
  - Matmuls run in fp32r (fp32 with 11-bit mantissa, 1 cycle/row on the
    PE vs 4 for plain fp32). Operands are pre-rounded on the host.
  - x is pre-transposed on the host to xT [I,B] packed as [128, 4, B]
    (I on partitions) so GEMM1 needs no on-chip transpose.
  - GEMM2's contraction is only K=17 (16 h rows + a ones row that pairs
    with a b2 row folded into the W2 operand, adding the bias for free).
    A 17-partition SBUF tensor would DMA-load through a single SDMA
    engine (~26 GB/s), so W2 is packed into FOUR row groups at partition
    bases 0/32/64/96, each holding a quarter of F -- a full-width fast
    DMA -- and GEMM2 issues row-group matmuls (tile_position=(32g,0)).
  - GEMM1 replicates hT into all 4 bases at no PE cost by replicating
    W1's output columns (M=128 instead of 16; matmul time is set by the
    moving dim only). Tanh+b1 is applied by ACT straight out of PSUM,
    once per row group.
  - PSUM [128,512] tiles -> SBUF via DVE/ACT copies (split 1:1) ->
    1 MiB HWDGE DMAs into the contiguous A / Bf halves.
"""

import sys

import numpy as np

try:
    import concourse  # noqa: F401
except ImportError:
    sys.path.insert(0, "/opt/trn_rl_repo")

E, B, I, O, R, H = 16, 1024, 512, 512, 8, 16
F = I * R + R * O  # 8192
N_CORES = 8
EL = E // N_CORES  # experts per core = 2
P = 128
KP = H + 1  # GEMM2 contraction rows per group: 16 h rows + 1 ones/bias row
NG = 4  # row groups (partition bases 0/32/64/96)
FG = F // NG  # 2048 F columns per row group
FC = 512  # matmul moving-dim chunk (one PSUM bank of fp32)
NC_G = FG // FC  # 4 chunks per group
NB = B // P  # 8 b-chunks
K1 = I // P  # 4 contraction chunks for GEMM1
FH = F // 2  # 4096, size of the A / Bf halves

_nc_cache = None


def _build_nc():
    import concourse.bacc as bacc
    import concourse.mybir as mybir
    import concourse.tile as tile

    f32 = mybir.dt.float32
    f32r = mybir.dt.float32r  # fp32 @ 1 cycle/row on the PE (11-bit mantissa)

    nc = bacc.Bacc(
        "TRN2",
        target_bir_lowering=False,
        debug=False,
        enable_asserts=False,
        num_devices=N_CORES,
    )

    xp_d = nc.dram_tensor("xp", (P, K1 * B), f32r, kind="ExternalInput")
    w1_d = nc.dram_tensor("w1p", (P, EL * K1 * P), f32r, kind="ExternalInput")
    b1_d = nc.dram_tensor("b1p", (P, EL), f32, kind="ExternalInput")
    w2_d = nc.dram_tensor("w2p", (P, EL * FG), f32r, kind="ExternalInput")
    pad_d = nc.dram_tensor("padp", (1, EL * B), f32r, kind="ExternalInput")
    a_d = nc.dram_tensor("a_out", (EL, B, FH), f32, kind="ExternalOutput")
    bf_d = nc.dram_tensor("bf_out", (EL, B, FH), f32, kind="ExternalOutput")

    with tile.TileContext(nc) as tc:
        with (
            tc.tile_pool(name="consts", bufs=1) as cpool,
            tc.tile_pool(name="fb", bufs=10) as fpool,
            tc.tile_pool(name="psum", bufs=8, space="PSUM") as pspool,
        ):
            # All input loads are full-width (128-partition) fast DMAs on
            # the sync ring, ordered so GEMM1 can start as early as possible.
            w1 = cpool.tile([P, EL, K1, P], f32r)
            w1_r = w1_d.ap().rearrange("p (e k m) -> p e k m", e=EL, k=K1)
            for e in range(EL):
                nc.sync.dma_start(w1[:, e], w1_r[:, e])
            b1s = cpool.tile([P, EL], f32)
            nc.sync.dma_start(b1s[:], b1_d.ap())
            xp_r = xp_d.ap().rearrange("p (k b) -> p k b", k=K1)
            xt = cpool.tile([P, K1, B], f32r)
            for k in range(K1):
                nc.sync.dma_start(xt[:, k], xp_r[:, k])
            w2 = cpool.tile([P, EL, FG], f32r)
            w2_r = w2_d.ap().rearrange("p (e f) -> p e f", e=EL)
            for e in range(EL):
                nc.sync.dma_start(w2[:, e], w2_r[:, e])

            ht = cpool.tile([P, EL, B], f32r)
            # Ones row at partition 32g+16 of each group (pairs with the b2
            # row of w2). Engine APs need 32-aligned partition bases, so
            # fill these single partitions via tiny DMAs (scalar ring, so
            # they don't queue behind the bulk loads on the sync ring).
            pad_r = pad_d.ap().rearrange("o (e b) -> o e b", e=EL)
            for g in range(NG):
                nc.gpsimd.dma_start(ht[32 * g + H : 32 * g + KP], pad_r)
            # Warm the ACT tanh table during the input-load phase so the
            # first real tanh doesn't pay the ~1.3us table load.
            warm = cpool.tile([1, 1], f32)
            nc.scalar.activation(
                warm[:], warm[:], mybir.ActivationFunctionType.Tanh
            )

            # GEMM1 + tanh: hT [16, B] per expert, replicated into the 4
            # row groups by w1's replicated output columns (single matmul
            # chain produces all 4 copies; ACT applies tanh+b1 per group).
            def gemm1(e, n):
                ph = pspool.tile([P, FC], f32, tag="bank", name=f"ph_{e}_{n}")
                for k in range(K1):
                    nc.tensor.matmul(
                        ph[:],
                        w1[:, e, k, :],
                        xt[:, k, n * FC : (n + 1) * FC],
                        start=(k == 0),
                        stop=(k == K1 - 1),
                    )
                for g in range(NG):
                    nc.scalar.activation(
                        ht[32 * g : 32 * g + H, e, n * FC : (n + 1) * FC],
                        ph[32 * g : 32 * g + H],
                        mybir.ActivationFunctionType.Tanh,
                        bias=b1s[32 * g : 32 * g + H, e : e + 1],
                    )

            # GEMM2: f[e, b-chunk] [128, F] via row-group matmuls; PSUM ->
            # SBUF copies split 1:1 over DVE/ACT; 1 MiB DMA per (group,
            # half-row-block) into the A (groups 0,1) / Bf (groups 2,3)
            # output halves. GEMM1 for each B-half is emitted just before
            # the b-chunks that consume it, so the first output tile's
            # chain is as short as possible and later GEMM1 work fills PE
            # gaps during GEMM2.
            for e in range(EL):
                for b in range(NB):
                    if b % (FC // P) == 0:
                        gemm1(e, b // (FC // P))
                    for g in range(NG):
                        fb = fpool.tile([P, FG], f32, tag="fb")
                        for c in range(NC_G):
                            pf = pspool.tile([P, FC], f32, tag="bank")
                            nc.tensor.matmul(
                                pf[:],
                                ht[32 * g : 32 * g + KP, e, b * P : (b + 1) * P],
                                w2[32 * g : 32 * g + KP, e, c * FC : (c + 1) * FC],
                                start=True,
                                stop=True,
                                tile_position=(32 * g, 0),
                            )
                            if c % 2 == 1:
                                nc.scalar.copy(pf_dst(fb, c), pf[:])
                            else:
                                nc.vector.tensor_copy(pf_dst(fb, c), pf[:])
                        out_d = a_d if g < 2 else bf_d
                        col0 = (g % 2) * FG
                        nc.sync.dma_start(
                            out_d.ap()[
                                e, b * P : (b + 1) * P, col0 : col0 + FG
                            ],
                            fb[:],
                        )

    nc.compile()
    return nc


def pf_dst(fb, c):
    return fb[:, c * FC : (c + 1) * FC]


def _round_fp32r(a):
    """Round fp32 to the PE's fp32r format (11 explicit mantissa bits,
    round-to-nearest-even) — matches walrus fp32_to_fp32r."""
    a = np.ascontiguousarray(a, dtype=np.float32)
    u = a.view(np.uint32)
    bias = ((u >> 12) & 1) + np.uint32(0x7FF)
    u2 = (u + bias) & np.uint32(0xFFFFF000)
    return u2.view(np.float32)


def _prep_inputs(x, W1, b1, W2, b2):
    """Host-side packing into the per-core DMA-friendly layouts."""
    x = np.ascontiguousarray(x, dtype=np.float32)
    W1 = np.ascontiguousarray(W1, dtype=np.float32)
    b1 = np.ascontiguousarray(b1, dtype=np.float32)
    W2 = np.ascontiguousarray(W2, dtype=np.float32)
    b2 = np.ascontiguousarray(b2, dtype=np.float32)
    # xT packed [P, K1*B]: xp[p, k*B + b] = x[b, k*128 + p]
    xp = _round_fp32r(
        np.ascontiguousarray(
            x.T.reshape(K1, P, B).transpose(1, 0, 2).reshape(P, K1 * B)
        )
    )
    padp = np.ones((1, EL * B), dtype=np.float32)
    in_maps = []
    for c in range(N_CORES):
        e0 = c * EL
        # w1 with output columns replicated into the 4 row groups:
        # w1p[p, e, k, 32g+h] = W1[e0+e, k*128+p, h]; cols 16..31 of each
        # group are zero (their hT rows are never read by GEMM2).
        w1b = W1[e0 : e0 + EL].reshape(EL, K1, P, H).transpose(2, 0, 1, 3)
        w1r = np.zeros((P, EL, K1, NG, 32), dtype=np.float32)
        w1r[..., :H] = w1b[:, :, :, None, :]
        w1p = _round_fp32r(w1r.reshape(P, EL * K1 * P))
        # b1 replicated per group: b1p[32g+h, e] = b1[e0+e, h]
        b1p = np.zeros((P, EL), dtype=np.float32)
        b1p.reshape(NG, 32, EL)[:, :H] = b1[e0 : e0 + EL].T[None, :, :]
        # w2 packed into row groups: w2p[32g+k, e, j] = W2[e0+e, k, g*FG+j]
        # (k<16); row 32g+16 = b2[e0+e, g*FG+j]; rows 17..31 unused.
        w2v = W2[e0 : e0 + EL].reshape(EL, H, NG, FG)
        w2c = np.zeros((NG, 32, EL, FG), dtype=np.float32)
        w2c[:, :H] = w2v.transpose(2, 1, 0, 3)
        w2c[:, H] = b2[e0 : e0 + EL].reshape(EL, NG, FG).transpose(1, 0, 2)
        w2p = _round_fp32r(w2c.reshape(P, EL * FG))
        in_maps.append(
            {"xp": xp, "w1p": w1p, "b1p": b1p, "w2p": w2p, "padp": padp}
        )
    return in_maps


def kernel(x, W1, b1, W2, b2, _want_results=False, **run_kwargs):
    global _nc_cache
    from concourse.bass_utils import run_bass_kernel_spmd

    if _nc_cache is None:
        _nc_cache = _build_nc()
    nc = _nc_cache

    in_maps = _prep_inputs(x, W1, b1, W2, b2)
    try:
        res = run_bass_kernel_spmd(
            nc, in_maps, core_ids=list(range(N_CORES)), **run_kwargs
        )
    except Exception:
        # Rare transient NRT execution failures recover on re-execution.
        res = run_bass_kernel_spmd(
            nc, in_maps, core_ids=list(range(N_CORES)), **run_kwargs
        )
    A = np.concatenate(
        [res.results[c]["a_out"].reshape(EL, B, I, R) for c in range(N_CORES)], axis=0
    )
    Bf = np.concatenate(
        [res.results[c]["bf_out"].reshape(EL, B, R, O) for c in range(N_CORES)], axis=0
    )
    if _want_results:
        return (A, Bf), res
    return (A, Bf)
